# revision 3
# baseline (speedup 1.0000x reference)
"""Trainium2 Bass kernel for nn_MlpwithSOMModule (sum-of-max hard-attention score).

Math identity: out[b,k] = sum_l max_m (ctx_l . ent_m)  -- no argmax/gather needed.
Sharding: B = 8 == n_cores; core c handles batch c (64 (b,k) pairs, 100.7 MB).

Per-core pipeline (Tile framework), v2:
  The v1 kernel (SWDGE fp32->bf16 cast-in-DMA) was bound by SDMA engine 15
  ("E79"), which sustains only ~21 GB/s HBM-side on the cast datapath vs
  ~26.8 GB/s for its 15 peers; since the SWDGE ucode balances bytes exactly
  evenly across the 16 engines, the whole stream completed at E79's pace:
  64 pairs x (98.3 KB / 21 GB/s) ~= 300 us.  Probes showed E79 matches its
  peers on the plain fp32 (no-cast) path, so v2:
    1. loads fp32 with NO dma cast (2 dma_starts per pair: ctx, ent --
       6 KB contiguous per-partition descriptors, tokens at l = 2p + r),
    2. casts fp32->bf16 on compute engines (DVE/GPSIMD, 4 chunks per pair,
       each gated only on its own dma so the PE wakes at half-pair
       granularity and HAM never sees a >3.4us idle),
    3. PE transposes 24 [128,128] bf16 blocks per pair to build ctxT/entT
       (d on partitions), ACT/DVE copy the PSUM slabs to SBUF,
    4. 12 accumulating bf16 matmuls per pair -> S[l,m]; DVE row-max; final
       sum over l via ones^T @ RM matmuls (split in half at pair 31).
  A burst of warmup matmuls on a zero tile runs during the first pair's
  load so the PE HAM clock-gate reaches K=8/8 before real work arrives.
"""

import sys

for _p in ("/opt/trn_rl_repo", "/root/.axon_site/_ro/trn_rl_repo"):
    if _p not in sys.path:
        sys.path.insert(0, _p)

import numpy as np

B, TOPK, L, D = 8, 64, 256, 768
N_CORES = 8
PAIRS = 64  # == TOPK; one batch index per core
P = 128
DCHUNKS = D // P  # 6
LCHUNKS = L // P  # 2

# Load strategy: which DGE issues the ctx / ent fp32 loads.
#   "swdge"  -> both via gpsimd (SWDGE)
#   "hwdge"  -> both via sync (HWDGE)
#   "mixed"  -> ctx via gpsimd, ent via sync (two independent ring sets)
LOAD = "swdge"
CAST_GP = True  # cast chunk 0 on gpsimd (else all chunks on DVE)
XBUFS = 10
WARM_MMS = 20

_cache = {}


def _build():
    import concourse.bass as bass
    import concourse.mybir as mybir
    from concourse import bacc
    from concourse.tile import TileContext
    from concourse.masks import make_identity

    nc = bacc.Bacc(
        "TRN2",
        target_bir_lowering=False,
        debug=False,
        num_devices=N_CORES,
    )

    x = nc.dram_tensor(
        "x", [PAIRS, 2, L, D], mybir.dt.float32, kind="ExternalInput"
    ).ap()
    out = nc.dram_tensor(
        "out", [1, PAIRS], mybir.dt.float32, kind="ExternalOutput"
    ).ap()

    bf16 = mybir.dt.bfloat16
    f32 = mybir.dt.float32

    # DRAM view: pair pr, slab s, partition p, row r (l = 2p + r), d.
    # Each partition reads 6 KB contiguous per slab. The token permutation
    # l = 2p + r is harmless: out = sum_l max_m is order-invariant.
    xv = x.rearrange("pr s (p two) d -> pr s p (two d)", p=P, two=2)

    with TileContext(nc) as tc:
        with (
            tc.tile_pool(name="const", bufs=1) as cpool,
            tc.tile_pool(name="xload", bufs=XBUFS) as xpool,
            tc.tile_pool(name="xcast", bufs=4) as bpool,
            tc.tile_pool(name="tpose", bufs=3) as tpool,
            tc.tile_pool(name="ppose", bufs=4, space="PSUM") as ppool,
            tc.tile_pool(name="pmm", bufs=3, space="PSUM") as mpool,
            tc.tile_pool(name="pfin", bufs=1, space="PSUM") as fpool,
        ):
            ident = cpool.tile([P, P], bf16)
            ones = cpool.tile([P, 1], f32)
            # row maxes: column 2*pair+lc holds max_m S[l, m] for l-chunk lc
            RM = cpool.tile([P, 2 * PAIRS], f32)
            wsb = cpool.tile([P, 256], bf16)

            # PE warmup: ~20 matmuls (N=256) on a zeroed tile keep the PE
            # busy through the HAM SHORT window while pair 0 loads, so real
            # matmuls start at K=8/8 (2.4 GHz) instead of cold 1.2 GHz.
            nc.gpsimd.memset(wsb, 0.0)
            wps = fpool.tile([P, 256], f32, tag="fin", name="wps")
            for _ in range(WARM_MMS):
                nc.tensor.matmul(wps, wsb[:, :P], wsb, start=True, stop=True)

            fin = fpool.tile([1, 2 * PAIRS], f32, tag="fin", name="fin")

            def emit_mm(pair, T):
                ps = mpool.tile([P, LCHUNKS, 2 * P], f32)
                for lc in range(LCHUNKS):
                    for dc in range(DCHUNKS):
                        off = (dc * 2 + lc) * P
                        nc.tensor.matmul(
                            ps[:, lc],
                            T[:, off : off + P],  # ctxT block [d, l-chunk]
                            T[:, 1536 + dc * 2 * P : 1536 + (dc + 1) * 2 * P],
                            start=(dc == 0),
                            stop=(dc == DCHUNKS - 1),
                        )
                nc.vector.reduce_max(
                    RM[:, 2 * pair : 2 * pair + 2], ps, axis=mybir.AxisListType.X
                )
                if pair == PAIRS // 2 - 1:
                    # first half of the final sum over l: only the second
                    # half remains on the critical path after the last pair
                    nc.tensor.matmul(
                        fin[:, :PAIRS],
                        ones,
                        RM[:, :PAIRS],
                        start=True,
                        stop=True,
                    )

            prev = None
            for q in range(PAIRS):
                X = xpool.tile([P, 4, D], f32, tag="X", name="X")
                Xb = bpool.tile([P, 4, D], bf16, tag="Xb", name="Xb")
                dma_ctx = nc.gpsimd if LOAD in ("swdge", "mixed") else nc.sync
                dma_ent = nc.gpsimd if LOAD == "swdge" else nc.sync
                dma_ctx.dma_start(
                    X[:, 0:2, :].rearrange("p c d -> p (c d)"), xv[q, 0]
                )
                dma_ent.dma_start(
                    X[:, 2:4, :].rearrange("p c d -> p (c d)"), xv[q, 1]
                )
                if q == 0:
                    # emitted after the first dma_starts so descriptor
                    # generation begins immediately at kernel start
                    make_identity(nc, ident)
                    nc.gpsimd.memset(ones, 1.0)
                if prev is not None:
                    emit_mm(*prev)

                # fp32 -> bf16 cast, one chunk per row-slot; each chunk only
                # waits on its own slab's dma. Emitted after emit_mm so the
                # previous pair's reduce_max sits ahead of the casts in the
                # DVE FIFO (casts gated on a late dma would otherwise
                # head-of-line-block it).
                for c in range(4):
                    eng = nc.gpsimd if (CAST_GP and c in (0, 2)) else nc.vector
                    eng.tensor_copy(Xb[:, c, :], X[:, c, :])

                # T: ctxT at [0, 1536), entT at [1536, 3072); block
                # (t, dc, lc) lives at free offset 128*(t*12 + dc*2 + lc)
                T = tpool.tile([P, 2 * 1536], bf16, tag="T", name="T")
                for jj in range(3):
                    psb = ppool.tile([P, 1024], bf16, tag="psb", name="psb")
                    for slot in range(8):
                        j = jj * 8 + slot
                        t, rem = divmod(j, 12)
                        dc, lc = divmod(rem, 2)
                        c = t * 2 + lc
                        nc.tensor.transpose(
                            psb[:, slot * P : (slot + 1) * P],
                            Xb[:, c, dc * P : (dc + 1) * P],
                            ident,
                        )
                    # 2:1 ACT/DVE split keeps DVE light (it also runs the
                    # casts and the per-pair reduce_max)
                    dst = T[:, jj * 1024 : (jj + 1) * 1024]
                    if jj == 1:
                        nc.vector.tensor_copy(dst, psb)
                    else:
                        nc.scalar.copy(dst, psb)
                prev = (q, T)

            emit_mm(*prev)

            # second half of out[pair] = sum over l (sum over 128 partitions)
            nc.tensor.matmul(
                fin[:, PAIRS:],
                ones,
                RM[:, PAIRS:],
                start=True,
                stop=True,
            )
            fsb = cpool.tile([1, 2 * PAIRS], f32)
            nc.vector.tensor_copy(fsb, fin)
            osb = cpool.tile([1, PAIRS], f32)
            fsb2 = fsb.rearrange("p (n two) -> p n two", two=2)
            nc.vector.tensor_tensor(
                osb, fsb2[:, :, 0], fsb2[:, :, 1], op=mybir.AluOpType.add
            )
            nc.sync.dma_start(out, osb)

    nc.compile()
    return nc


def _get_nc():
    if "nc" not in _cache:
        _cache["nc"] = _build()
    return _cache["nc"]


def run(context, trace=False, tmpdir=None):
    from concourse import bass_utils

    nc = _get_nc()
    context = np.ascontiguousarray(np.asarray(context, dtype=np.float32))
    assert context.shape == (B, TOPK, 2, L, D), context.shape
    in_maps = [{"x": context[c]} for c in range(N_CORES)]
    res = bass_utils.run_bass_kernel_spmd(
        nc, in_maps, core_ids=list(range(N_CORES)), trace=trace, tmpdir=tmpdir
    )
    out = np.concatenate(
        [res.results[c]["out"].reshape(1, PAIRS) for c in range(N_CORES)],
        axis=0,
    ).astype(np.float32)
    return out, res


def kernel(context):
    out, _ = run(context, trace=False)
    return out


# revision 6
# speedup vs baseline: 1.9023x; 1.9023x over previous
"""Trainium2 Bass kernel for nn_MlpwithSOMModule (sum-of-max hard-attention score).

Math identity: out[b,k] = sum_l max_m (ctx_l . ent_m)  -- no argmax/gather needed.
Sharding: B = 8 == n_cores; core c handles batch c (64 (b,k) pairs, 100.7 MB).

Per-core pipeline (Tile framework), v2:
  The v1 kernel (SWDGE fp32->bf16 cast-in-DMA) was bound by SDMA engine 15
  ("E79"), which sustains only ~21 GB/s HBM-side on the cast datapath vs
  ~26.8 GB/s for its 15 peers; since the SWDGE ucode balances bytes exactly
  evenly across the 16 engines, the whole stream completed at E79's pace:
  64 pairs x (98.3 KB / 21 GB/s) ~= 300 us.  Probes showed E79 matches its
  peers on the plain fp32 (no-cast) path, so v2:
    1. loads fp32 with NO dma cast (2 dma_starts per pair: ctx, ent --
       6 KB contiguous per-partition descriptors, tokens at l = 2p + r),
    2. casts fp32->bf16 on compute engines (DVE/GPSIMD, 4 chunks per pair,
       each gated only on its own dma so the PE wakes at half-pair
       granularity and HAM never sees a >3.4us idle),
    3. PE transposes 24 [128,128] bf16 blocks per pair to build ctxT/entT
       (d on partitions), ACT/DVE copy the PSUM slabs to SBUF,
    4. 12 accumulating bf16 matmuls per pair -> S[l,m]; DVE row-max; final
       sum over l via ones^T @ RM matmuls (split in half at pair 31).
  A burst of warmup matmuls on a zero tile runs during the first pair's
  load so the PE HAM clock-gate reaches K=8/8 before real work arrives.
"""

import sys

for _p in ("/opt/trn_rl_repo", "/root/.axon_site/_ro/trn_rl_repo"):
    if _p not in sys.path:
        sys.path.insert(0, _p)

import numpy as np

B, TOPK, L, D = 8, 64, 256, 768
N_CORES = 8
PAIRS = 64  # == TOPK; one batch index per core
P = 128
DCHUNKS = D // P  # 6
LCHUNKS = L // P  # 2

# Load strategy: which DGE issues the ctx / ent fp32 loads.
#   "swdge"  -> both via gpsimd (SWDGE)
#   "hwdge"  -> both via sync (HWDGE)
#   "mixed"  -> ctx via gpsimd, ent via sync (two independent ring sets)
LOAD = "swdge"
XBUFS = 10
WARM_MMS = 20

_cache = {}


def _build():
    import concourse.bass as bass
    import concourse.mybir as mybir
    from concourse import bacc
    from concourse.tile import TileContext
    from concourse.masks import make_identity

    nc = bacc.Bacc(
        "TRN2",
        target_bir_lowering=False,
        debug=False,
        num_devices=N_CORES,
    )

    x = nc.dram_tensor(
        "x", [PAIRS, 2, L, D], mybir.dt.float32, kind="ExternalInput"
    ).ap()
    out = nc.dram_tensor(
        "out", [1, PAIRS], mybir.dt.float32, kind="ExternalOutput"
    ).ap()

    bf16 = mybir.dt.bfloat16
    f32 = mybir.dt.float32

    # DRAM view: pair pr, slab s, partition p, row r (l = 2p + r), d.
    # Each partition reads 6 KB contiguous per slab. The token permutation
    # l = 2p + r is harmless: out = sum_l max_m is order-invariant.
    xv = x.rearrange("pr s (p two) d -> pr s p (two d)", p=P, two=2)

    with TileContext(nc) as tc:
        with (
            tc.tile_pool(name="const", bufs=1) as cpool,
            tc.tile_pool(name="xload", bufs=XBUFS) as xpool,
            tc.tile_pool(name="xcast", bufs=4) as bpool,
            tc.tile_pool(name="tpose", bufs=3) as tpool,
            tc.tile_pool(name="ppose", bufs=4, space="PSUM") as ppool,
            tc.tile_pool(name="pmm", bufs=3, space="PSUM") as mpool,
            tc.tile_pool(name="pfin", bufs=1, space="PSUM") as fpool,
        ):
            ident = cpool.tile([P, P], bf16)
            ones = cpool.tile([P, 1], f32)
            # row maxes: column 2*pair+lc holds max_m S[l, m] for l-chunk lc
            RM = cpool.tile([P, 2 * PAIRS], f32)
            wsb = cpool.tile([P, 256], bf16)

            # PE warmup: ~20 matmuls (N=256) on a zeroed tile keep the PE
            # busy through the HAM SHORT window while pair 0 loads, so real
            # matmuls start at K=8/8 (2.4 GHz) instead of cold 1.2 GHz.
            nc.gpsimd.memset(wsb, 0.0)
            wps = fpool.tile([P, 256], f32, tag="fin", name="wps")
            for _ in range(WARM_MMS):
                nc.tensor.matmul(wps, wsb[:, :P], wsb, start=True, stop=True)

            fin = fpool.tile([1, 2 * PAIRS], f32, tag="fin", name="fin")

            def emit_mm(pair, T):
                ps = mpool.tile([P, LCHUNKS, 2 * P], f32)
                for lc in range(LCHUNKS):
                    for dc in range(DCHUNKS):
                        off = (dc * 2 + lc) * P
                        nc.tensor.matmul(
                            ps[:, lc],
                            T[:, off : off + P],  # ctxT block [d, l-chunk]
                            T[:, 1536 + dc * 2 * P : 1536 + (dc + 1) * 2 * P],
                            start=(dc == 0),
                            stop=(dc == DCHUNKS - 1),
                        )
                nc.vector.reduce_max(
                    RM[:, 2 * pair : 2 * pair + 2], ps, axis=mybir.AxisListType.X
                )
                if pair == PAIRS // 2 - 1:
                    # first half of the final sum over l: only the second
                    # half remains on the critical path after the last pair
                    nc.tensor.matmul(
                        fin[:, :PAIRS],
                        ones,
                        RM[:, :PAIRS],
                        start=True,
                        stop=True,
                    )

            prev = None
            for q in range(PAIRS):
                X = xpool.tile([P, 4, D], f32, tag="X", name="X")
                Xb = bpool.tile([P, 4, D], bf16, tag="Xb", name="Xb")
                dma_ctx = nc.gpsimd if LOAD in ("swdge", "mixed") else nc.sync
                dma_ent = nc.gpsimd if LOAD == "swdge" else nc.sync
                dma_ctx.dma_start(
                    X[:, 0:2, :].rearrange("p c d -> p (c d)"), xv[q, 0]
                )
                dma_ent.dma_start(
                    X[:, 2:4, :].rearrange("p c d -> p (c d)"), xv[q, 1]
                )
                if q == 0:
                    # emitted after the first dma_starts so descriptor
                    # generation begins immediately at kernel start
                    make_identity(nc, ident)
                    nc.gpsimd.memset(ones, 1.0)
                if prev is not None:
                    emit_mm(*prev)

                # fp32 -> bf16 cast on ACT (~1.09 ns/elem measured; DVE casts
                # run at only ~1.94 ns/elem and GPSIMD ~3.75, so ACT does all
                # of it and DVE gets the PSUM slab copies instead). One chunk
                # per slab, each gated only on its own dma.
                for c in range(2):
                    nc.scalar.copy(
                        Xb[:, 2 * c : 2 * c + 2, :].rearrange("p c d -> p (c d)"),
                        X[:, 2 * c : 2 * c + 2, :].rearrange("p c d -> p (c d)"),
                    )

                # T: ctxT at [0, 1536), entT at [1536, 3072); block
                # (t, dc, lc) lives at free offset 128*(t*12 + dc*2 + lc)
                T = tpool.tile([P, 2 * 1536], bf16, tag="T", name="T")
                for jj in range(3):
                    psb = ppool.tile([P, 1024], bf16, tag="psb", name="psb")
                    for slot in range(8):
                        j = jj * 8 + slot
                        t, rem = divmod(j, 12)
                        dc, lc = divmod(rem, 2)
                        c = t * 2 + lc
                        nc.tensor.transpose(
                            psb[:, slot * P : (slot + 1) * P],
                            Xb[:, c, dc * P : (dc + 1) * P],
                            ident,
                        )
                    # all slab copies on DVE (ACT is saturated by the casts)
                    dst = T[:, jj * 1024 : (jj + 1) * 1024]
                    nc.vector.tensor_copy(dst, psb)
                prev = (q, T)

            emit_mm(*prev)

            # second half of out[pair] = sum over l (sum over 128 partitions)
            nc.tensor.matmul(
                fin[:, PAIRS:],
                ones,
                RM[:, PAIRS:],
                start=True,
                stop=True,
            )
            fsb = cpool.tile([1, 2 * PAIRS], f32)
            nc.vector.tensor_copy(fsb, fin)
            osb = cpool.tile([1, PAIRS], f32)
            fsb2 = fsb.rearrange("p (n two) -> p n two", two=2)
            nc.vector.tensor_tensor(
                osb, fsb2[:, :, 0], fsb2[:, :, 1], op=mybir.AluOpType.add
            )
            nc.sync.dma_start(out, osb)

    nc.compile()
    return nc


def _get_nc():
    if "nc" not in _cache:
        _cache["nc"] = _build()
    return _cache["nc"]


def run(context, trace=False, tmpdir=None):
    from concourse import bass_utils

    nc = _get_nc()
    context = np.ascontiguousarray(np.asarray(context, dtype=np.float32))
    assert context.shape == (B, TOPK, 2, L, D), context.shape
    in_maps = [{"x": context[c]} for c in range(N_CORES)]
    res = bass_utils.run_bass_kernel_spmd(
        nc, in_maps, core_ids=list(range(N_CORES)), trace=trace, tmpdir=tmpdir
    )
    out = np.concatenate(
        [res.results[c]["out"].reshape(1, PAIRS) for c in range(N_CORES)],
        axis=0,
    ).astype(np.float32)
    return out, res


def kernel(context):
    out, _ = run(context, trace=False)
    return out


# revision 7
# speedup vs baseline: 1.9075x; 1.0027x over previous
"""Trainium2 Bass kernel for nn_MlpwithSOMModule (sum-of-max hard-attention score).

Math identity: out[b,k] = sum_l max_m (ctx_l . ent_m)  -- no argmax/gather needed.
Sharding: B = 8 == n_cores; core c handles batch c (64 (b,k) pairs, 100.7 MB).

Per-core pipeline (Tile framework), v2:
  The v1 kernel (SWDGE fp32->bf16 cast-in-DMA) was bound by SDMA engine 15
  ("E79"), which sustains only ~21 GB/s HBM-side on the cast datapath vs
  ~26.8 GB/s for its 15 peers; since the SWDGE ucode balances bytes exactly
  evenly across the 16 engines, the whole stream completed at E79's pace:
  64 pairs x (98.3 KB / 21 GB/s) ~= 300 us.  Probes showed E79 matches its
  peers on the plain fp32 (no-cast) path, so v2:
    1. loads fp32 with NO dma cast (2 dma_starts per pair: ctx, ent --
       6 KB contiguous per-partition descriptors, tokens at l = 2p + r),
    2. casts fp32->bf16 on compute engines (DVE/GPSIMD, 4 chunks per pair,
       each gated only on its own dma so the PE wakes at half-pair
       granularity and HAM never sees a >3.4us idle),
    3. PE transposes 24 [128,128] bf16 blocks per pair to build ctxT/entT
       (d on partitions), ACT/DVE copy the PSUM slabs to SBUF,
    4. 12 accumulating bf16 matmuls per pair -> S[l,m]; DVE row-max; final
       sum over l via ones^T @ RM matmuls (split in half at pair 31).
  A burst of warmup matmuls on a zero tile runs during the first pair's
  load so the PE HAM clock-gate reaches K=8/8 before real work arrives.
"""

import sys

for _p in ("/opt/trn_rl_repo", "/root/.axon_site/_ro/trn_rl_repo"):
    if _p not in sys.path:
        sys.path.insert(0, _p)

import numpy as np

B, TOPK, L, D = 8, 64, 256, 768
N_CORES = 8
PAIRS = 64  # == TOPK; one batch index per core
P = 128
DCHUNKS = D // P  # 6
LCHUNKS = L // P  # 2

# Load strategy: which DGE issues the ctx / ent fp32 loads.
#   "swdge"  -> both via gpsimd (SWDGE)
#   "hwdge"  -> both via sync (HWDGE)
#   "mixed"  -> ctx via gpsimd, ent via sync (two independent ring sets)
LOAD = "swdge"
XBUFS = 10
WARM_MMS = 20

_cache = {}


def _build():
    import concourse.bass as bass
    import concourse.mybir as mybir
    from concourse import bacc
    from concourse.tile import TileContext
    from concourse.masks import make_identity

    nc = bacc.Bacc(
        "TRN2",
        target_bir_lowering=False,
        debug=False,
        num_devices=N_CORES,
    )

    x = nc.dram_tensor(
        "x", [PAIRS, 2, L, D], mybir.dt.float32, kind="ExternalInput"
    ).ap()
    out = nc.dram_tensor(
        "out", [1, PAIRS], mybir.dt.float32, kind="ExternalOutput"
    ).ap()

    bf16 = mybir.dt.bfloat16
    f32 = mybir.dt.float32

    # DRAM view: pair pr, slab s, partition p, row r (l = 2p + r), d.
    # Each partition reads 6 KB contiguous per slab. The token permutation
    # l = 2p + r is harmless: out = sum_l max_m is order-invariant.
    xv = x.rearrange("pr s (p two) d -> pr s p (two d)", p=P, two=2)

    with TileContext(nc) as tc:
        with (
            tc.tile_pool(name="const", bufs=1) as cpool,
            tc.tile_pool(name="xload", bufs=XBUFS) as xpool,
            tc.tile_pool(name="xcast", bufs=8) as bpool,
            tc.tile_pool(name="tpose", bufs=4) as tpool,
            tc.tile_pool(name="ppose", bufs=4, space="PSUM") as ppool,
            tc.tile_pool(name="pmm", bufs=3, space="PSUM") as mpool,
            tc.tile_pool(name="pfin", bufs=1, space="PSUM") as fpool,
        ):
            ident = cpool.tile([P, P], bf16)
            ones = cpool.tile([P, 1], f32)
            # row maxes: column 2*pair+lc holds max_m S[l, m] for l-chunk lc
            RM = cpool.tile([P, 2 * PAIRS], f32)
            wsb = cpool.tile([P, 256], bf16)

            # PE warmup: ~20 matmuls (N=256) on a zeroed tile keep the PE
            # busy through the HAM SHORT window while pair 0 loads, so real
            # matmuls start at K=8/8 (2.4 GHz) instead of cold 1.2 GHz.
            nc.gpsimd.memset(wsb, 0.0)
            wps = fpool.tile([P, 256], f32, tag="fin", name="wps")
            for _ in range(WARM_MMS):
                nc.tensor.matmul(wps, wsb[:, :P], wsb, start=True, stop=True)

            fin = fpool.tile([1, 2 * PAIRS], f32, tag="fin", name="fin")

            def emit_mm(pair, T):
                ps = mpool.tile([P, LCHUNKS, 2 * P], f32)
                for lc in range(LCHUNKS):
                    for dc in range(DCHUNKS):
                        off = (dc * 2 + lc) * P
                        nc.tensor.matmul(
                            ps[:, lc],
                            T[:, off : off + P],  # ctxT block [d, l-chunk]
                            T[:, 1536 + dc * 2 * P : 1536 + (dc + 1) * 2 * P],
                            start=(dc == 0),
                            stop=(dc == DCHUNKS - 1),
                        )
                nc.vector.reduce_max(
                    RM[:, 2 * pair : 2 * pair + 2], ps, axis=mybir.AxisListType.X
                )
                if pair == PAIRS // 2 - 1:
                    # first half of the final sum over l: only the second
                    # half remains on the critical path after the last pair
                    nc.tensor.matmul(
                        fin[:, :PAIRS],
                        ones,
                        RM[:, :PAIRS],
                        start=True,
                        stop=True,
                    )

            prev = None
            for q in range(PAIRS):
                X = xpool.tile([P, 4, D], f32, tag="X", name="X")
                Xb = bpool.tile([P, 4, D], bf16, tag="Xb", name="Xb")
                dma_ctx = nc.gpsimd if LOAD in ("swdge", "mixed") else nc.sync
                dma_ent = nc.gpsimd if LOAD == "swdge" else nc.sync
                dma_ctx.dma_start(
                    X[:, 0:2, :].rearrange("p c d -> p (c d)"), xv[q, 0]
                )
                dma_ent.dma_start(
                    X[:, 2:4, :].rearrange("p c d -> p (c d)"), xv[q, 1]
                )
                if q == 0:
                    # emitted after the first dma_starts so descriptor
                    # generation begins immediately at kernel start
                    make_identity(nc, ident)
                    nc.gpsimd.memset(ones, 1.0)
                if prev is not None:
                    emit_mm(*prev)

                # fp32 -> bf16 cast on ACT (~1.09 ns/elem measured; DVE casts
                # run at only ~1.94 ns/elem and GPSIMD ~3.75, so ACT does all
                # of it and DVE gets the PSUM slab copies instead). One chunk
                # per slab, each gated only on its own dma.
                for c in range(2):
                    nc.scalar.copy(
                        Xb[:, 2 * c : 2 * c + 2, :].rearrange("p c d -> p (c d)"),
                        X[:, 2 * c : 2 * c + 2, :].rearrange("p c d -> p (c d)"),
                    )

                # T: ctxT at [0, 1536), entT at [1536, 3072); block
                # (t, dc, lc) lives at free offset 128*(t*12 + dc*2 + lc)
                T = tpool.tile([P, 2 * 1536], bf16, tag="T", name="T")
                for jj in range(3):
                    psb = ppool.tile([P, 1024], bf16, tag="psb", name="psb")
                    for slot in range(8):
                        j = jj * 8 + slot
                        t, rem = divmod(j, 12)
                        dc, lc = divmod(rem, 2)
                        c = t * 2 + lc
                        nc.tensor.transpose(
                            psb[:, slot * P : (slot + 1) * P],
                            Xb[:, c, dc * P : (dc + 1) * P],
                            ident,
                        )
                    # all slab copies on DVE (ACT is saturated by the casts)
                    dst = T[:, jj * 1024 : (jj + 1) * 1024]
                    nc.vector.tensor_copy(dst, psb)
                prev = (q, T)

            emit_mm(*prev)

            # second half of out[pair] = sum over l (sum over 128 partitions)
            nc.tensor.matmul(
                fin[:, PAIRS:],
                ones,
                RM[:, PAIRS:],
                start=True,
                stop=True,
            )
            fsb = cpool.tile([1, 2 * PAIRS], f32)
            nc.vector.tensor_copy(fsb, fin)
            osb = cpool.tile([1, PAIRS], f32)
            fsb2 = fsb.rearrange("p (n two) -> p n two", two=2)
            nc.vector.tensor_tensor(
                osb, fsb2[:, :, 0], fsb2[:, :, 1], op=mybir.AluOpType.add
            )
            nc.sync.dma_start(out, osb)

    nc.compile()
    return nc


def _get_nc():
    if "nc" not in _cache:
        _cache["nc"] = _build()
    return _cache["nc"]


def run(context, trace=False, tmpdir=None):
    from concourse import bass_utils

    nc = _get_nc()
    context = np.ascontiguousarray(np.asarray(context, dtype=np.float32))
    assert context.shape == (B, TOPK, 2, L, D), context.shape
    in_maps = [{"x": context[c]} for c in range(N_CORES)]
    res = bass_utils.run_bass_kernel_spmd(
        nc, in_maps, core_ids=list(range(N_CORES)), trace=trace, tmpdir=tmpdir
    )
    out = np.concatenate(
        [res.results[c]["out"].reshape(1, PAIRS) for c in range(N_CORES)],
        axis=0,
    ).astype(np.float32)
    return out, res


def kernel(context):
    out, _ = run(context, trace=False)
    return out
